# revision 25
# baseline (speedup 1.0000x reference)
"""ImprovedGRUCell Trainium2 kernel v3 (8-core data-parallel over batch).

Design notes (vs the f32 baseline):
  - x / h_prev converted to bf16 on HOST -> input DMA traffic halved.
    h_prev is used in bf16 everywhere including the final blend.
  - fp8-e4m3 DoubleRow matmuls (full K=256 contraction in one pass, 0.5
    cycles/row) for the z-gate and attention-branch gemms; W_h / U_h
    (candidate state, the precision-critical path) stay bf16.
  - Softmax normalization: dB = ones128^T @ E8 (DoubleRow) broadcasts the
    per-column denominators to every PSUM partition; R = 1/dB (DVE),
    q = h^T * R, att = E8 * q -- so the U_h gemm consumes pre-normalized
    bf16 operands and accumulates into the SAME psum group as W_h's
    output; tanh reads one [128,1024] psum tile. No per-j scalar fixups.
  - 5-deep software pipeline over 512-row super-tiles (load+2, xpose+1,
    gemms+act, normalize+candidate-1, blend-2) so every in-order engine
    queue only ever waits on work from previous iterations.
  - GPSIMD (Pool) never touches PSUM (hardware restriction); all psum
    drains are DVE (2x mode for bf16) or ACT.
  - ACT does exactly 4 transcendental passes per super-tile with
    {Tanh, Exp} in one table set (z via 0.5*tanh(S/2)+0.5, the /2 folded
    into the activation scale).
  - PSUM: pz(2 banks) + pa(2) + pc(2) + transposes(2x1) = 8 banks.
"""

import os
import sys

sys.path.insert(0, "/opt/trn_rl_repo")

import ml_dtypes
import numpy as np

import concourse.bass as bass
import concourse.mybir as mybir
from concourse import bacc, tile
from concourse.bass_utils import run_bass_kernel_spmd

B_TOTAL = 65536
N_CORES = 8
B_CORE = B_TOTAL // N_CORES  # 8192
D = 256
ST = 512  # batch rows per super-tile
N_ST = B_CORE // ST  # 16

F32 = mybir.dt.float32
BF16 = mybir.dt.bfloat16
F8 = mybir.dt.float8e4
AF = mybir.ActivationFunctionType
ALU = mybir.AluOpType
DR = mybir.MatmulPerfMode.DoubleRow

# z-gate gemm precision: "full8" = both operands fp8 DoubleRow (fastest),
# "half" = x-side fp8-DR + h-side bf16 (better accuracy, PE still under the
# ACT bound), "bf16" = both sides bf16 (max accuracy, PE-bound)
ZMODE = os.environ.get("ZMODE", "half")
Z8 = ZMODE != "bf16"

_CACHE = {}


def build_nc(use_bias=False):
    nc = bacc.Bacc(
        "TRN2",
        target_bir_lowering=False,
        debug=False,
        enable_asserts=False,
        num_devices=N_CORES,
    )

    x_d = nc.dram_tensor("x", [B_CORE, D], BF16, kind="ExternalInput")
    h_d = nc.dram_tensor("h", [B_CORE, D], BF16, kind="ExternalInput")
    # weights in [128 p, kt 2, 256 h] contraction layout, host-packed
    wz_d = nc.dram_tensor("wz", [128, 2 * D], BF16, kind="ExternalInput")
    uz_d = nc.dram_tensor("uz", [128, 2 * D], BF16, kind="ExternalInput")
    wz8_d = nc.dram_tensor("wz8", [128, 2 * D], F8, kind="ExternalInput")
    uz8_d = nc.dram_tensor("uz8", [128, 2 * D], F8, kind="ExternalInput")
    wh_d = nc.dram_tensor("wh", [128, 2 * D], BF16, kind="ExternalInput")
    uh_d = nc.dram_tensor("uh", [128, 2 * D], BF16, kind="ExternalInput")
    uh8_d = nc.dram_tensor("uh8", [128, 2 * D], F8, kind="ExternalInput")
    wa_d = nc.dram_tensor("wa", [128, 2 * D], F8, kind="ExternalInput")
    ua_d = nc.dram_tensor("ua", [128, 2 * D], F8, kind="ExternalInput")
    va_d = nc.dram_tensor("va", [128, 2], F32, kind="ExternalInput")
    vad_d = nc.dram_tensor("vad", [128, 512], F8, kind="ExternalInput")
    id_d = nc.dram_tensor("ident", [128, 128], BF16, kind="ExternalInput")
    bz_d = nc.dram_tensor("bz", [1, D], BF16, kind="ExternalInput")
    bh_d = nc.dram_tensor("bh", [1, D], BF16, kind="ExternalInput")
    out_d = nc.dram_tensor("out", [B_CORE, D], F32, kind="ExternalOutput")

    with tile.TileContext(nc) as tc:
        with (
            tc.tile_pool(name="wp", bufs=1) as wp,
            tc.tile_pool(name="io", bufs=4) as io,
            tc.tile_pool(name="wk", bufs=3) as wk,
            tc.tile_pool(name="ppz", bufs=1, space="PSUM") as ppz,
            tc.tile_pool(name="ppa", bufs=1, space="PSUM") as ppa,
            tc.tile_pool(name="ppc", bufs=1, space="PSUM") as ppc,
            tc.tile_pool(name="pxp", bufs=2, space="PSUM") as pxp,
        ):
            # ---- persistent weights -------------------------------------
            # (ident + the first two activation loads are emitted before the
            # weight DMAs so the transpose pipeline fills ASAP; see below)
            def wload(name, dram, dt):
                t = wp.tile([128, 2 * D], dt, tag=name)
                nc.sync.dma_start(out=t[:], in_=dram.ap())
                return t.rearrange("p (kt h) -> p kt h", kt=2)

            ident = wp.tile([128, 128], BF16, tag="ident")
            nc.sync.dma_start(out=ident[:], in_=id_d.ap())
            ones8 = wp.tile([128, 2 * 128], F8, tag="ones8")
            nc.vector.memset(ones8[:], 1.0 / 16)
            ones8v = ones8.rearrange("p (kt m) -> p kt m", kt=2)
            if use_bias:
                bz = wp.tile([1, D], BF16, tag="bz")
                nc.sync.dma_start(out=bz[:], in_=bz_d.ap())
                bh = wp.tile([1, D], BF16, tag="bh")
                nc.sync.dma_start(out=bh[:], in_=bh_d.ap())
                ones_r = wp.tile([1, 128], BF16, tag="ones_r")
                nc.vector.memset(ones_r[:], 1.0)

            S = {}  # per-super-tile tile handles across pipeline stages

            def load(i):
                b0 = i * ST
                xb = io.tile([128, 4 * D], BF16, tag="xb", bufs=3)
                nc.sync.dma_start(
                    out=xb.rearrange("p (j k) -> p j k", j=4),
                    in_=x_d.ap()[b0 : b0 + ST, :].rearrange("(j p) k -> p j k", p=128),
                )
                hb = io.tile([128, 4 * D], BF16, tag="hb", bufs=7)
                nc.sync.dma_start(
                    out=hb.rearrange("p (j k) -> p j k", j=4),
                    in_=h_d.ap()[b0 : b0 + ST, :].rearrange("(j p) k -> p j k", p=128),
                )
                S[i] = dict(xb=xb, hb=hb)

            def xpose(i):
                # [p=b, j, k] -> [p=k, kt, b]  (kt-major, DoubleRow layout)
                s = S[i]

                def one(src3, tag):
                    pt = pxp.tile([128, 4 * D], BF16, tag="pxp")
                    pt3 = pt.rearrange("p (kt b) -> p kt b", kt=2)
                    for kt in range(2):
                        for j in range(4):
                            nc.tensor.transpose(
                                pt3[:, kt, j * 128 : (j + 1) * 128],
                                src3[:, j, kt * 128 : (kt + 1) * 128],
                                ident[:],
                            )
                    t16 = wk.tile([128, 4 * D], BF16, tag=tag, bufs=6)
                    nc.vector.tensor_copy(t16[:], pt[:])
                    t8 = wk.tile([128, 4 * D], F8, tag=tag + "8", bufs=4)
                    nc.gpsimd.tensor_copy(t8[:], t16[:])
                    return t16, t8

                s["xT"], s["xT8"] = one(s["xb"].rearrange("p (j k) -> p j k", j=4), "xT")
                s["hT"], s["hT8"] = one(s["hb"].rearrange("p (j k) -> p j k", j=4), "hT")

            def gemmsA(i):
                s = S[i]
                xT83 = s["xT8"].rearrange("p (kt b) -> p kt b", kt=2)
                hT83 = s["hT8"].rearrange("p (kt b) -> p kt b", kt=2)

                # ---- z-gate gemm: psum = S_z (+ b_z) --------------------
                pz = ppz.tile([128, 4 * D], F32, tag="ppz")
                pz3 = pz.rearrange("p (j h) -> p j h", j=4)
                zmode_i = "bf16" if (ZMODE == "half" and i == 0) else ZMODE
                if zmode_i == "full8":
                    for j in range(4):
                        nc.tensor.matmul(
                            pz3[:, j],
                            xT83[:, :, j * 128 : (j + 1) * 128],
                            wz8[:, :, :],
                            start=True,
                            stop=False,
                            perf_mode=DR,
                        )
                        nc.tensor.matmul(
                            pz3[:, j],
                            hT83[:, :, j * 128 : (j + 1) * 128],
                            uz8[:, :, :],
                            start=False,
                            stop=not use_bias,
                            perf_mode=DR,
                            skip_group_check=True,
                        )
                        if use_bias:
                            nc.tensor.matmul(
                                pz3[:, j], ones_r[:], bz[:], start=False, stop=True,
                                skip_group_check=True,
                            )
                elif zmode_i == "half":
                    # x-side fp8-DR (weights host-scaled x8); h-side bf16.
                    # bf16 U_z is host-scaled x8 too so both sides share the
                    # tanh input scale of 0.5/8.
                    hT3h = s["hT"].rearrange("p (kt b) -> p kt b", kt=2)
                    for j in range(4):
                        nc.tensor.matmul(
                            pz3[:, j],
                            xT83[:, :, j * 128 : (j + 1) * 128],
                            wz8[:, :, :],
                            start=True,
                            stop=False,
                            perf_mode=DR,
                        )
                        for kt in range(2):
                            nc.tensor.matmul(
                                pz3[:, j],
                                hT3h[:, kt, j * 128 : (j + 1) * 128],
                                uz[:, kt],
                                start=False,
                                stop=(kt == 1 and not use_bias),
                                skip_group_check=True,
                            )
                        if use_bias:
                            nc.tensor.matmul(
                                pz3[:, j], ones_r[:], bz[:], start=False, stop=True,
                                skip_group_check=True,
                            )
                else:
                    xT3 = s["xT"].rearrange("p (kt b) -> p kt b", kt=2)
                    hT3 = s["hT"].rearrange("p (kt b) -> p kt b", kt=2)
                    n_pass = 5 if use_bias else 4
                    for j in range(4):
                        k = 0
                        for src, w3 in ((xT3, wz), (hT3, uz)):
                            for kt in range(2):
                                nc.tensor.matmul(
                                    pz3[:, j],
                                    src[:, kt, j * 128 : (j + 1) * 128],
                                    w3[:, kt],
                                    start=(k == 0),
                                    stop=(k == n_pass - 1),
                                )
                                k += 1
                        if use_bias:
                            nc.tensor.matmul(
                                pz3[:, j], ones_r[:], bz[:], start=False, stop=True
                            )
                # t = tanh((S_z+b_z)/2); z = 0.5*t+0.5 later
                t_t = wk.tile([128, 4 * D], BF16, tag="t", bufs=5)
                # fp8 weights are host-scaled by 8 (keeps them in e4m3's
                # normal range); fold the 1/8 into the tanh input scale
                nc.scalar.activation(
                    t_t[:], pz[:], AF.Tanh, scale=(0.5 / 8 if Z8 else 0.5)
                )

                # ---- attention gemms (fp8 DoubleRow), transposed --------
                pa = ppa.tile([128, 4 * D], F32, tag="ppa")
                pa3 = pa.rearrange("p (ht b) -> p ht b", ht=2)
                for ht in range(2):
                    nc.tensor.matmul(
                        pa3[:, ht],
                        wa[:, :, ht * 128 : (ht + 1) * 128],
                        xT83[:, :, :],
                        start=True,
                        stop=False,
                        perf_mode=DR,
                    )
                    nc.tensor.matmul(
                        pa3[:, ht],
                        ua[:, :, ht * 128 : (ht + 1) * 128],
                        hT83[:, :, :],
                        start=False,
                        stop=True,
                        perf_mode=DR,
                    )
                A = wk.tile([128, 4 * D], F8, tag="A", bufs=2)
                A3 = A.rearrange("p (ht b) -> p ht b", ht=2)
                nc.scalar.activation(A[:], pa[:], AF.Tanh, scale=1.0 / 8)
                E8 = wk.tile([128, 4 * D], F8, tag="E8", bufs=3)
                if i == 0:
                    # during pipeline fill the ACT->PE->ACT round trip below
                    # would stall ACT; use the direct per-ht exp once
                    for ht in range(2):
                        nc.scalar.activation(
                            E8[:, ht * 512 : (ht + 1) * 512],
                            A[:, ht * 512 : (ht + 1) * 512],
                            AF.Exp,
                            scale=va[:, ht : ht + 1],
                        )
                else:
                    # u = diag(va) (x) A via fp8 DoubleRow, overwriting pa;
                    # then a single full-width exp
                    for ht in range(2):
                        nc.tensor.matmul(
                            pa3[:, ht],
                            vad4[:, ht],
                            A3[:, :, :],
                            start=True,
                            stop=True,
                            perf_mode=DR,
                            skip_group_check=True,
                        )
                    nc.scalar.activation(E8[:], pa[:], AF.Exp)
                s.update(E8=E8, t=t_t)

            def stageB1(i):
                s = S[i]
                # broadcast softmax denominators: every psum partition gets
                # d[b] = sum_h E8[h,b]; lives in the transpose-psum ring,
                # which is idle at this point of the iteration
                dBt = pxp.tile([128, 512], F32, tag="pxp")
                dB = dBt[:, :]
                E83 = s["E8"].rearrange("p (ht b) -> p ht b", ht=2)
                nc.tensor.matmul(
                    dB,
                    ones8v[:, :, :],
                    E83[:, :, :],
                    start=True,
                    stop=True,
                    perf_mode=DR,
                    skip_group_check=True,
                )
                R = wk.tile([128, 512], BF16, tag="R", bufs=2)
                with nc.allow_low_precision(reason="1/denominator to bf16 is fine"):
                    nc.vector.reciprocal(R[:], dB)
                q = wk.tile([128, 4 * D], BF16, tag="q", bufs=2)
                q3 = q.rearrange("p (kt b) -> p kt b", kt=2)
                hT3 = s["hT"].rearrange("p (kt b) -> p kt b", kt=2)
                att = wk.tile([128, 4 * D], F8, tag="att", bufs=3)
                # att^T = E8 * (h^T / d)  -- normalized, bf16.  The last
                # tile runs in b-halves so the epilogue chain pipelines.
                halves = 2 if i == N_ST - 1 else 1
                bw = 512 // halves
                for hh in range(halves):
                    bsl = slice(hh * bw, (hh + 1) * bw)
                    for kt in range(2):
                        nc.vector.tensor_mul(
                            q3[:, kt, bsl], hT3[:, kt, bsl], R[:, bsl]
                        )
                    if halves == 1:
                        nc.gpsimd.tensor_mul(att[:], s["E8"][:], q[:])
                    else:
                        att3 = att.rearrange("p (kt b) -> p kt b", kt=2)
                        E83f = s["E8"].rearrange("p (kt b) -> p kt b", kt=2)
                        for kt in range(2):
                            nc.gpsimd.tensor_mul(
                                att3[:, kt, bsl], E83f[:, kt, bsl], q3[:, kt, bsl]
                            )
                s["att"] = att

            def stageB2(i):
                s = S[i]
                att3 = s["att"].rearrange("p (kt b) -> p kt b", kt=2)
                xT3 = s["xT"].rearrange("p (kt b) -> p kt b", kt=2)

                # candidate: psum = x@W_h^T + att@U_h^T (+ b_h), per-j group
                # (the final tile borrows the z psum banks, idle by then, so
                # the last two candidate gemms don't serialize on one ring)
                pool_c = ppz if i == N_ST - 1 else ppc
                tag_c = "ppz" if i == N_ST - 1 else "ppc"
                pc = pool_c.tile([128, 4 * D], F32, tag=tag_c)
                pc3 = pc.rearrange("p (j h) -> p j h", j=4)
                for j in range(4):
                    for kt in range(2):
                        nc.tensor.matmul(
                            pc3[:, j],
                            xT3[:, kt, j * 128 : (j + 1) * 128],
                            wh[:, kt],
                            start=(kt == 0),
                            stop=False,
                        )
                    nc.tensor.matmul(
                        pc3[:, j],
                        att3[:, :, j * 128 : (j + 1) * 128],
                        uh8[:, :, :],
                        start=False,
                        stop=not use_bias,
                        perf_mode=DR,
                        skip_group_check=True,
                    )
                    if use_bias:
                        nc.tensor.matmul(
                            pc3[:, j], ones_r[:], bh[:], start=False, stop=True,
                            skip_group_check=True,
                        )
                htl = wk.tile([128, 4 * D], BF16, tag="htl", bufs=3)
                if i == N_ST - 1:
                    nc.scalar.activation(htl[:, 0:512], pc[:, 0:512], AF.Tanh, scale=1 / 16)
                    nc.scalar.activation(htl[:, 512:1024], pc[:, 512:1024], AF.Tanh, scale=1 / 16)
                else:
                    nc.scalar.activation(htl[:], pc[:], AF.Tanh, scale=1 / 16)
                s["htl"] = htl

            def stageC(i):
                s = S.pop(i)
                b0 = i * ST
                zb = wk.tile([128, 4 * D], BF16, tag="zb")
                if i != N_ST - 1:
                    nc.vector.tensor_scalar(
                        zb[:], s["t"][:], 0.5, 0.5, op0=ALU.mult, op1=ALU.add
                    )
                ot = io.tile([128, 4 * D], F32, tag="ot", bufs=3)
                # the last tile drains in halves so DVE/Pool/DMA pipeline
                # behind the final tanh instead of serializing the epilogue
                halves = 2 if i == N_ST - 1 else 1
                w = (4 * D) // halves
                rows = ST // halves
                for hh in range(halves):
                    sl = slice(hh * w, (hh + 1) * w)
                    if i == N_ST - 1:
                        nc.vector.tensor_scalar(
                            zb[:, sl], s["t"][:, sl], 0.5, 0.5,
                            op0=ALU.mult, op1=ALU.add,
                        )
                    d = wk.tile([128, w], BF16, tag=f"d{hh}")
                    nc.vector.tensor_sub(d[:], s["htl"][:, sl], s["hb"][:, sl])
                    p = wk.tile([128, w], BF16, tag=f"p{hh}")
                    nc.vector.tensor_mul(p[:], zb[:, sl], d[:])
                    nc.gpsimd.tensor_add(ot[:, sl], p[:], s["hb"][:, sl])
                    r0 = b0 + hh * rows
                    nc.sync.dma_start(
                        out=out_d.ap()[r0 : r0 + rows, :].rearrange(
                            "(j p) k -> p j k", p=128
                        ),
                        in_=ot[:, sl].rearrange("p (j k) -> p j k", j=4 // halves),
                    )

            load(0)
            if N_ST > 1:
                load(1)
            xpose(0)
            wz = wload("wz", wz_d, BF16) if ZMODE == "bf16" else None
            uz = wload("uz", uz_d, BF16) if ZMODE in ("bf16", "half") else None
            wz8 = wload("wz8", wz8_d, F8) if Z8 else None
            uz8 = wload("uz8", uz8_d, F8) if ZMODE == "full8" else None
            wh = wload("wh", wh_d, BF16)  # host-scaled x16
            uh8 = wload("uh8", uh8_d, F8)
            wa = wload("wa", wa_d, F8)
            ua = wload("ua", ua_d, F8)
            vad = wp.tile([128, 512], F8, tag="vad")
            nc.sync.dma_start(out=vad[:], in_=vad_d.ap())
            vad4 = vad.rearrange("p (sel t m) -> p sel t m", sel=2, t=2)
            va = wp.tile([128, 2], F32, tag="va")
            nc.sync.dma_start(out=va[:], in_=va_d.ap())
            if ZMODE == "half":
                wz = wload("wz", wz_d, BF16)  # tile-0 bf16 z, x8-scaled
            if N_ST > 1:
                xpose(1)
            for it in range(N_ST + 3):
                if it + 2 < N_ST:
                    load(it + 2)
                if it < N_ST:
                    gemmsA(it)
                if 0 <= it - 2 < N_ST:
                    stageB2(it - 2)
                if it + 2 < N_ST:
                    xpose(it + 2)
                if it < N_ST:
                    stageB1(it)
                if 0 <= it - 3 < N_ST:
                    stageC(it - 3)

    nc.compile()
    return nc


LAST_RESULTS = None


def _pack_vad(v_a):
    # diag(va) as DoubleRow lhsT pair: [p, sel(2), t(2), m(128)];
    # sel=ht selects which half of va sits on the diagonal (t == sel)
    f8 = ml_dtypes.float8_e4m3
    v = np.asarray(v_a, dtype=np.float32)
    Dv = np.zeros((128, 2, 2, 128), dtype=np.float32)
    idx = np.arange(128)
    Dv[idx, 0, 0, idx] = v[:128]
    Dv[idx, 1, 1, idx] = v[128:]
    return np.ascontiguousarray(Dv.reshape(128, 512).astype(f8))


def _pack_w(W, dt):
    # [out,in] weight -> contraction layout [128 p, kt 2, 256 out] flat
    WT = np.asarray(W, dtype=np.float32).T  # [in 256, out 256]
    return np.ascontiguousarray(
        WT.reshape(2, 128, D).transpose(1, 0, 2).reshape(128, 2 * D).astype(dt)
    )


def kernel(x, h_prev, W_z, U_z, b_z, W_a, U_a, v_a, W_h, U_h, b_h):
    global LAST_RESULTS
    use_bias = bool(np.any(np.asarray(b_z)) or np.any(np.asarray(b_h)))
    key = ("nc", use_bias)
    if key not in _CACHE:
        _CACHE[key] = build_nc(use_bias)
    nc = _CACHE[key]

    bf = ml_dtypes.bfloat16
    f8 = ml_dtypes.float8_e4m3
    x = np.asarray(x, dtype=np.float32).astype(bf)
    h_prev = np.asarray(h_prev, dtype=np.float32).astype(bf)

    common = {
        "wz": _pack_w(
            np.asarray(W_z) * (8.0 if ZMODE == "half" else 1.0), bf
        ),
        "uz": _pack_w(
            np.asarray(U_z) * (8.0 if ZMODE == "half" else 1.0), bf
        ),
        "wz8": _pack_w(np.asarray(W_z) * 8.0, f8),
        "uz8": _pack_w(np.asarray(U_z) * 8.0, f8),
        "wh": _pack_w(np.asarray(W_h) * 16.0, bf),
        "uh": _pack_w(U_h, bf),
        "uh8": _pack_w(U_h, f8),
        "wa": _pack_w(np.asarray(W_a) * 8.0, f8),
        "ua": _pack_w(np.asarray(U_a) * 8.0, f8),
        "va": np.ascontiguousarray(
            np.asarray(v_a, dtype=np.float32).reshape(2, 128).T
        ),
        "vad": _pack_vad(v_a),
        "ident": np.eye(128, dtype=bf),
        "bz": (np.asarray(b_z, dtype=np.float32) * (8.0 if Z8 else 1.0))
        .reshape(1, D)
        .astype(bf),
        "bh": (np.asarray(b_h, dtype=np.float32) * 16.0).reshape(1, D).astype(bf),
    }

    in_maps = []
    for c in range(N_CORES):
        m = dict(common)
        m["x"] = x[c * B_CORE : (c + 1) * B_CORE]
        m["h"] = h_prev[c * B_CORE : (c + 1) * B_CORE]
        in_maps.append(m)

    LAST_RESULTS = run_bass_kernel_spmd(nc, in_maps, core_ids=list(range(N_CORES)))
    outs = LAST_RESULTS.results
    return np.concatenate([outs[c]["out"] for c in range(N_CORES)], axis=0)


# revision 26
# speedup vs baseline: 1.0098x; 1.0098x over previous
"""ImprovedGRUCell Trainium2 kernel v3 (8-core data-parallel over batch).

Design notes (vs the f32 baseline):
  - x / h_prev converted to bf16 on HOST -> input DMA traffic halved.
    h_prev is used in bf16 everywhere including the final blend.
  - fp8-e4m3 DoubleRow matmuls (full K=256 contraction in one pass, 0.5
    cycles/row) for the z-gate and attention-branch gemms; W_h / U_h
    (candidate state, the precision-critical path) stay bf16.
  - Softmax normalization: dB = ones128^T @ E8 (DoubleRow) broadcasts the
    per-column denominators to every PSUM partition; R = 1/dB (DVE),
    q = h^T * R, att = E8 * q -- so the U_h gemm consumes pre-normalized
    bf16 operands and accumulates into the SAME psum group as W_h's
    output; tanh reads one [128,1024] psum tile. No per-j scalar fixups.
  - 5-deep software pipeline over 512-row super-tiles (load+2, xpose+1,
    gemms+act, normalize+candidate-1, blend-2) so every in-order engine
    queue only ever waits on work from previous iterations.
  - GPSIMD (Pool) never touches PSUM (hardware restriction); all psum
    drains are DVE (2x mode for bf16) or ACT.
  - ACT does exactly 4 transcendental passes per super-tile with
    {Tanh, Exp} in one table set (z via 0.5*tanh(S/2)+0.5, the /2 folded
    into the activation scale).
  - PSUM: pz(2 banks) + pa(2) + pc(2) + transposes(2x1) = 8 banks.
"""

import os
import sys

sys.path.insert(0, "/opt/trn_rl_repo")

import ml_dtypes
import numpy as np

import concourse.bass as bass
import concourse.mybir as mybir
from concourse import bacc, tile
from concourse.bass_utils import run_bass_kernel_spmd

B_TOTAL = 65536
N_CORES = 8
B_CORE = B_TOTAL // N_CORES  # 8192
D = 256
ST = 512  # batch rows per super-tile
N_ST = B_CORE // ST  # 16

F32 = mybir.dt.float32
BF16 = mybir.dt.bfloat16
F8 = mybir.dt.float8e4
AF = mybir.ActivationFunctionType
ALU = mybir.AluOpType
DR = mybir.MatmulPerfMode.DoubleRow

# z-gate gemm precision: "full8" = both operands fp8 DoubleRow (fastest),
# "half" = x-side fp8-DR + h-side bf16 (better accuracy, PE still under the
# ACT bound), "bf16" = both sides bf16 (max accuracy, PE-bound)
ZMODE = os.environ.get("ZMODE", "half")
Z8 = ZMODE != "bf16"

_CACHE = {}


def build_nc(use_bias=False):
    nc = bacc.Bacc(
        "TRN2",
        target_bir_lowering=False,
        debug=False,
        enable_asserts=False,
        num_devices=N_CORES,
    )

    x_d = nc.dram_tensor("x", [B_CORE, D], BF16, kind="ExternalInput")
    h_d = nc.dram_tensor("h", [B_CORE, D], BF16, kind="ExternalInput")
    # weights in [128 p, kt 2, 256 h] contraction layout, host-packed
    wz_d = nc.dram_tensor("wz", [128, 2 * D], BF16, kind="ExternalInput")
    uz_d = nc.dram_tensor("uz", [128, 2 * D], BF16, kind="ExternalInput")
    wz8_d = nc.dram_tensor("wz8", [128, 2 * D], F8, kind="ExternalInput")
    uz8_d = nc.dram_tensor("uz8", [128, 2 * D], F8, kind="ExternalInput")
    wh_d = nc.dram_tensor("wh", [128, 2 * D], BF16, kind="ExternalInput")
    uh_d = nc.dram_tensor("uh", [128, 2 * D], BF16, kind="ExternalInput")
    uh8_d = nc.dram_tensor("uh8", [128, 2 * D], F8, kind="ExternalInput")
    wa_d = nc.dram_tensor("wa", [128, 2 * D], F8, kind="ExternalInput")
    ua_d = nc.dram_tensor("ua", [128, 2 * D], F8, kind="ExternalInput")
    va_d = nc.dram_tensor("va", [128, 2], F32, kind="ExternalInput")
    vad_d = nc.dram_tensor("vad", [128, 512], F8, kind="ExternalInput")
    id_d = nc.dram_tensor("ident", [128, 128], BF16, kind="ExternalInput")
    bz_d = nc.dram_tensor("bz", [1, D], BF16, kind="ExternalInput")
    bh_d = nc.dram_tensor("bh", [1, D], BF16, kind="ExternalInput")
    out_d = nc.dram_tensor("out", [B_CORE, D], F32, kind="ExternalOutput")

    with tile.TileContext(nc) as tc:
        with (
            tc.tile_pool(name="wp", bufs=1) as wp,
            tc.tile_pool(name="io", bufs=4) as io,
            tc.tile_pool(name="wk", bufs=3) as wk,
            tc.tile_pool(name="ppz", bufs=1, space="PSUM") as ppz,
            tc.tile_pool(name="ppa", bufs=1, space="PSUM") as ppa,
            tc.tile_pool(name="ppc", bufs=1, space="PSUM") as ppc,
            tc.tile_pool(name="pxp", bufs=2, space="PSUM") as pxp,
        ):
            # ---- persistent weights -------------------------------------
            # (ident + the first two activation loads are emitted before the
            # weight DMAs so the transpose pipeline fills ASAP; see below)
            def wload(name, dram, dt):
                t = wp.tile([128, 2 * D], dt, tag=name)
                nc.sync.dma_start(out=t[:], in_=dram.ap())
                return t.rearrange("p (kt h) -> p kt h", kt=2)

            ident = wp.tile([128, 128], BF16, tag="ident")
            nc.sync.dma_start(out=ident[:], in_=id_d.ap())
            ones8 = wp.tile([128, 2 * 128], F8, tag="ones8")
            nc.vector.memset(ones8[:], 1.0 / 16)
            ones8v = ones8.rearrange("p (kt m) -> p kt m", kt=2)
            if use_bias:
                bz = wp.tile([1, D], BF16, tag="bz")
                nc.sync.dma_start(out=bz[:], in_=bz_d.ap())
                bh = wp.tile([1, D], BF16, tag="bh")
                nc.sync.dma_start(out=bh[:], in_=bh_d.ap())
                ones_r = wp.tile([1, 128], BF16, tag="ones_r")
                nc.vector.memset(ones_r[:], 1.0)

            S = {}  # per-super-tile tile handles across pipeline stages

            def load(i):
                b0 = i * ST
                xb = io.tile([128, 4 * D], BF16, tag="xb", bufs=3)
                nc.sync.dma_start(
                    out=xb.rearrange("p (j k) -> p j k", j=4),
                    in_=x_d.ap()[b0 : b0 + ST, :].rearrange("(j p) k -> p j k", p=128),
                )
                hb = io.tile([128, 4 * D], BF16, tag="hb", bufs=7)
                nc.sync.dma_start(
                    out=hb.rearrange("p (j k) -> p j k", j=4),
                    in_=h_d.ap()[b0 : b0 + ST, :].rearrange("(j p) k -> p j k", p=128),
                )
                S[i] = dict(xb=xb, hb=hb)

            def xpose(i):
                # [p=b, j, k] -> [p=k, kt, b]  (kt-major, DoubleRow layout)
                s = S[i]

                def one(src3, tag):
                    pt = pxp.tile([128, 4 * D], BF16, tag="pxp")
                    pt3 = pt.rearrange("p (kt b) -> p kt b", kt=2)
                    for kt in range(2):
                        for j in range(4):
                            nc.tensor.transpose(
                                pt3[:, kt, j * 128 : (j + 1) * 128],
                                src3[:, j, kt * 128 : (kt + 1) * 128],
                                ident[:],
                            )
                    t16 = wk.tile([128, 4 * D], BF16, tag=tag, bufs=6)
                    nc.vector.tensor_copy(t16[:], pt[:])
                    t8 = wk.tile([128, 4 * D], F8, tag=tag + "8", bufs=4)
                    nc.gpsimd.tensor_copy(t8[:], t16[:])
                    return t16, t8

                s["xT"], s["xT8"] = one(s["xb"].rearrange("p (j k) -> p j k", j=4), "xT")
                s["hT"], s["hT8"] = one(s["hb"].rearrange("p (j k) -> p j k", j=4), "hT")

            def gemmsA(i):
                s = S[i]
                xT83 = s["xT8"].rearrange("p (kt b) -> p kt b", kt=2)
                hT83 = s["hT8"].rearrange("p (kt b) -> p kt b", kt=2)

                # ---- z-gate gemm: psum = S_z (+ b_z) --------------------
                pz = ppz.tile([128, 4 * D], F32, tag="ppz")
                pz3 = pz.rearrange("p (j h) -> p j h", j=4)
                zmode_i = "bf16" if (ZMODE == "half" and i == 0) else ZMODE
                if zmode_i == "full8":
                    for j in range(4):
                        nc.tensor.matmul(
                            pz3[:, j],
                            xT83[:, :, j * 128 : (j + 1) * 128],
                            wz8[:, :, :],
                            start=True,
                            stop=False,
                            perf_mode=DR,
                        )
                        nc.tensor.matmul(
                            pz3[:, j],
                            hT83[:, :, j * 128 : (j + 1) * 128],
                            uz8[:, :, :],
                            start=False,
                            stop=not use_bias,
                            perf_mode=DR,
                            skip_group_check=True,
                        )
                        if use_bias:
                            nc.tensor.matmul(
                                pz3[:, j], ones_r[:], bz[:], start=False, stop=True,
                                skip_group_check=True,
                            )
                elif zmode_i == "half":
                    # x-side fp8-DR (weights host-scaled x8); h-side bf16.
                    # bf16 U_z is host-scaled x8 too so both sides share the
                    # tanh input scale of 0.5/8.
                    hT3h = s["hT"].rearrange("p (kt b) -> p kt b", kt=2)
                    for j in range(4):
                        nc.tensor.matmul(
                            pz3[:, j],
                            xT83[:, :, j * 128 : (j + 1) * 128],
                            wz8[:, :, :],
                            start=True,
                            stop=False,
                            perf_mode=DR,
                        )
                        for kt in range(2):
                            nc.tensor.matmul(
                                pz3[:, j],
                                hT3h[:, kt, j * 128 : (j + 1) * 128],
                                uz[:, kt],
                                start=False,
                                stop=(kt == 1 and not use_bias),
                                skip_group_check=True,
                            )
                        if use_bias:
                            nc.tensor.matmul(
                                pz3[:, j], ones_r[:], bz[:], start=False, stop=True,
                                skip_group_check=True,
                            )
                else:
                    xT3 = s["xT"].rearrange("p (kt b) -> p kt b", kt=2)
                    hT3 = s["hT"].rearrange("p (kt b) -> p kt b", kt=2)
                    n_pass = 5 if use_bias else 4
                    for j in range(4):
                        k = 0
                        for src, w3 in ((xT3, wz), (hT3, uz)):
                            for kt in range(2):
                                nc.tensor.matmul(
                                    pz3[:, j],
                                    src[:, kt, j * 128 : (j + 1) * 128],
                                    w3[:, kt],
                                    start=(k == 0),
                                    stop=(k == n_pass - 1),
                                )
                                k += 1
                        if use_bias:
                            nc.tensor.matmul(
                                pz3[:, j], ones_r[:], bz[:], start=False, stop=True
                            )
                # t = tanh((S_z+b_z)/2); z = 0.5*t+0.5 later
                t_t = wk.tile([128, 4 * D], BF16, tag="t", bufs=5)
                # fp8 weights are host-scaled by 8 (keeps them in e4m3's
                # normal range); fold the 1/8 into the tanh input scale
                nc.scalar.activation(
                    t_t[:], pz[:], AF.Tanh, scale=(0.5 / 8 if Z8 else 0.5)
                )

                # ---- attention gemms (fp8 DoubleRow), transposed --------
                pa = ppa.tile([128, 4 * D], F32, tag="ppa")
                pa3 = pa.rearrange("p (ht b) -> p ht b", ht=2)
                for ht in range(2):
                    nc.tensor.matmul(
                        pa3[:, ht],
                        wa[:, :, ht * 128 : (ht + 1) * 128],
                        xT83[:, :, :],
                        start=True,
                        stop=False,
                        perf_mode=DR,
                    )
                    nc.tensor.matmul(
                        pa3[:, ht],
                        ua[:, :, ht * 128 : (ht + 1) * 128],
                        hT83[:, :, :],
                        start=False,
                        stop=True,
                        perf_mode=DR,
                    )
                A = wk.tile([128, 4 * D], F8, tag="A", bufs=2)
                A3 = A.rearrange("p (ht b) -> p ht b", ht=2)
                nc.scalar.activation(A[:], pa[:], AF.Tanh, scale=1.0 / 8)
                E8 = wk.tile([128, 4 * D], F8, tag="E8", bufs=3)
                if i == 0:
                    # during pipeline fill the ACT->PE->ACT round trip below
                    # would stall ACT; use the direct per-ht exp once
                    for ht in range(2):
                        nc.scalar.activation(
                            E8[:, ht * 512 : (ht + 1) * 512],
                            A[:, ht * 512 : (ht + 1) * 512],
                            AF.Exp,
                            scale=va[:, ht : ht + 1],
                        )
                else:
                    # u = diag(va) (x) A via fp8 DoubleRow, overwriting pa;
                    # then a single full-width exp
                    for ht in range(2):
                        nc.tensor.matmul(
                            pa3[:, ht],
                            vad4[:, ht],
                            A3[:, :, :],
                            start=True,
                            stop=True,
                            perf_mode=DR,
                            skip_group_check=True,
                        )
                    nc.scalar.activation(E8[:], pa[:], AF.Exp)
                s.update(E8=E8, t=t_t)

            def stageB1(i):
                s = S[i]
                # broadcast softmax denominators: every psum partition gets
                # d[b] = sum_h E8[h,b]; lives in the transpose-psum ring,
                # which is idle at this point of the iteration
                dBt = pxp.tile([128, 512], F32, tag="pxp")
                dB = dBt[:, :]
                E83 = s["E8"].rearrange("p (ht b) -> p ht b", ht=2)
                nc.tensor.matmul(
                    dB,
                    ones8v[:, :, :],
                    E83[:, :, :],
                    start=True,
                    stop=True,
                    perf_mode=DR,
                    skip_group_check=True,
                )
                R = wk.tile([128, 512], BF16, tag="R", bufs=2)
                with nc.allow_low_precision(reason="1/denominator to bf16 is fine"):
                    nc.vector.reciprocal(R[:], dB)
                q = wk.tile([128, 4 * D], BF16, tag="q", bufs=2)
                q3 = q.rearrange("p (kt b) -> p kt b", kt=2)
                hT3 = s["hT"].rearrange("p (kt b) -> p kt b", kt=2)
                att = wk.tile([128, 4 * D], F8, tag="att", bufs=3)
                # att^T = E8 * (h^T / d)  -- normalized, bf16.  The last
                # tile runs in b-halves so the epilogue chain pipelines.
                halves = 2 if i == N_ST - 1 else 1
                bw = 512 // halves
                for hh in range(halves):
                    bsl = slice(hh * bw, (hh + 1) * bw)
                    for kt in range(2):
                        nc.vector.tensor_mul(
                            q3[:, kt, bsl], hT3[:, kt, bsl], R[:, bsl]
                        )
                    if halves == 1:
                        nc.gpsimd.tensor_mul(att[:], s["E8"][:], q[:])
                    else:
                        att3 = att.rearrange("p (kt b) -> p kt b", kt=2)
                        E83f = s["E8"].rearrange("p (kt b) -> p kt b", kt=2)
                        for kt in range(2):
                            nc.gpsimd.tensor_mul(
                                att3[:, kt, bsl], E83f[:, kt, bsl], q3[:, kt, bsl]
                            )
                s["att"] = att

            def stageB2(i):
                s = S[i]
                att3 = s["att"].rearrange("p (kt b) -> p kt b", kt=2)
                xT3 = s["xT"].rearrange("p (kt b) -> p kt b", kt=2)

                # candidate: psum = x@W_h^T + att@U_h^T (+ b_h), per-j group
                # (the final tile borrows the z psum banks, idle by then, so
                # the last two candidate gemms don't serialize on one ring)
                pool_c = ppz if i == N_ST - 1 else ppc
                tag_c = "ppz" if i == N_ST - 1 else "ppc"
                pc = pool_c.tile([128, 4 * D], F32, tag=tag_c)
                pc3 = pc.rearrange("p (j h) -> p j h", j=4)
                for j in range(4):
                    for kt in range(2):
                        nc.tensor.matmul(
                            pc3[:, j],
                            xT3[:, kt, j * 128 : (j + 1) * 128],
                            wh[:, kt],
                            start=(kt == 0),
                            stop=False,
                        )
                    nc.tensor.matmul(
                        pc3[:, j],
                        att3[:, :, j * 128 : (j + 1) * 128],
                        uh8[:, :, :],
                        start=False,
                        stop=not use_bias,
                        perf_mode=DR,
                        skip_group_check=True,
                    )
                    if use_bias:
                        nc.tensor.matmul(
                            pc3[:, j], ones_r[:], bh[:], start=False, stop=True,
                            skip_group_check=True,
                        )
                htl = wk.tile([128, 4 * D], BF16, tag="htl", bufs=3)
                if i == N_ST - 1:
                    nc.scalar.activation(htl[:, 0:512], pc[:, 0:512], AF.Tanh, scale=1 / 16)
                    nc.scalar.activation(htl[:, 512:1024], pc[:, 512:1024], AF.Tanh, scale=1 / 16)
                else:
                    nc.scalar.activation(htl[:], pc[:], AF.Tanh, scale=1 / 16)
                s["htl"] = htl

            def stageC(i):
                s = S.pop(i)
                b0 = i * ST
                zb = wk.tile([128, 4 * D], BF16, tag="zb")
                if i != N_ST - 1:
                    nc.vector.tensor_scalar(
                        zb[:], s["t"][:], 0.5, 0.5, op0=ALU.mult, op1=ALU.add
                    )
                ot = io.tile([128, 4 * D], F32, tag="ot", bufs=3)
                # the last tile drains in halves so DVE/Pool/DMA pipeline
                # behind the final tanh instead of serializing the epilogue
                halves = 2 if i == N_ST - 1 else 1
                w = (4 * D) // halves
                rows = ST // halves
                for hh in range(halves):
                    sl = slice(hh * w, (hh + 1) * w)
                    if i == N_ST - 1:
                        nc.vector.tensor_scalar(
                            zb[:, sl], s["t"][:, sl], 0.5, 0.5,
                            op0=ALU.mult, op1=ALU.add,
                        )
                    d = wk.tile([128, w], BF16, tag=f"d{hh}")
                    nc.vector.tensor_sub(d[:], s["htl"][:, sl], s["hb"][:, sl])
                    p = wk.tile([128, w], BF16, tag=f"p{hh}")
                    nc.vector.tensor_mul(p[:], zb[:, sl], d[:])
                    nc.gpsimd.tensor_add(ot[:, sl], p[:], s["hb"][:, sl])
                    r0 = b0 + hh * rows
                    nc.sync.dma_start(
                        out=out_d.ap()[r0 : r0 + rows, :].rearrange(
                            "(j p) k -> p j k", p=128
                        ),
                        in_=ot[:, sl].rearrange("p (j k) -> p j k", j=4 // halves),
                    )

            load(0)
            # wz/uz gate tile 0's bf16 z-gemm (the first ACT op): load them
            # right after tile 0's data, before the tile-1 prefetch (which
            # isn't consumed until a full iteration later)
            wz = wload("wz", wz_d, BF16) if ZMODE in ("bf16", "half") else None
            uz = wload("uz", uz_d, BF16) if ZMODE in ("bf16", "half") else None
            if N_ST > 1:
                load(1)
            xpose(0)
            wz8 = wload("wz8", wz8_d, F8) if Z8 else None
            uz8 = wload("uz8", uz8_d, F8) if ZMODE == "full8" else None
            wh = wload("wh", wh_d, BF16)  # host-scaled x16
            uh8 = wload("uh8", uh8_d, F8)
            wa = wload("wa", wa_d, F8)
            ua = wload("ua", ua_d, F8)
            vad = wp.tile([128, 512], F8, tag="vad")
            nc.sync.dma_start(out=vad[:], in_=vad_d.ap())
            vad4 = vad.rearrange("p (sel t m) -> p sel t m", sel=2, t=2)
            va = wp.tile([128, 2], F32, tag="va")
            nc.sync.dma_start(out=va[:], in_=va_d.ap())
            if N_ST > 1:
                xpose(1)
            for it in range(N_ST + 3):
                if it + 2 < N_ST:
                    load(it + 2)
                if it < N_ST:
                    gemmsA(it)
                if 0 <= it - 2 < N_ST:
                    stageB2(it - 2)
                if it + 2 < N_ST:
                    xpose(it + 2)
                if it < N_ST:
                    stageB1(it)
                if 0 <= it - 3 < N_ST:
                    stageC(it - 3)

    nc.compile()
    return nc


LAST_RESULTS = None


def _pack_vad(v_a):
    # diag(va) as DoubleRow lhsT pair: [p, sel(2), t(2), m(128)];
    # sel=ht selects which half of va sits on the diagonal (t == sel)
    f8 = ml_dtypes.float8_e4m3
    v = np.asarray(v_a, dtype=np.float32)
    Dv = np.zeros((128, 2, 2, 128), dtype=np.float32)
    idx = np.arange(128)
    Dv[idx, 0, 0, idx] = v[:128]
    Dv[idx, 1, 1, idx] = v[128:]
    return np.ascontiguousarray(Dv.reshape(128, 512).astype(f8))


def _pack_w(W, dt):
    # [out,in] weight -> contraction layout [128 p, kt 2, 256 out] flat
    WT = np.asarray(W, dtype=np.float32).T  # [in 256, out 256]
    return np.ascontiguousarray(
        WT.reshape(2, 128, D).transpose(1, 0, 2).reshape(128, 2 * D).astype(dt)
    )


def kernel(x, h_prev, W_z, U_z, b_z, W_a, U_a, v_a, W_h, U_h, b_h):
    global LAST_RESULTS
    use_bias = bool(np.any(np.asarray(b_z)) or np.any(np.asarray(b_h)))
    key = ("nc", use_bias)
    if key not in _CACHE:
        _CACHE[key] = build_nc(use_bias)
    nc = _CACHE[key]

    bf = ml_dtypes.bfloat16
    f8 = ml_dtypes.float8_e4m3
    x = np.asarray(x, dtype=np.float32).astype(bf)
    h_prev = np.asarray(h_prev, dtype=np.float32).astype(bf)

    common = {
        "wz": _pack_w(
            np.asarray(W_z) * (8.0 if ZMODE == "half" else 1.0), bf
        ),
        "uz": _pack_w(
            np.asarray(U_z) * (8.0 if ZMODE == "half" else 1.0), bf
        ),
        "wz8": _pack_w(np.asarray(W_z) * 8.0, f8),
        "uz8": _pack_w(np.asarray(U_z) * 8.0, f8),
        "wh": _pack_w(np.asarray(W_h) * 16.0, bf),
        "uh": _pack_w(U_h, bf),
        "uh8": _pack_w(U_h, f8),
        "wa": _pack_w(np.asarray(W_a) * 8.0, f8),
        "ua": _pack_w(np.asarray(U_a) * 8.0, f8),
        "va": np.ascontiguousarray(
            np.asarray(v_a, dtype=np.float32).reshape(2, 128).T
        ),
        "vad": _pack_vad(v_a),
        "ident": np.eye(128, dtype=bf),
        "bz": (np.asarray(b_z, dtype=np.float32) * (8.0 if Z8 else 1.0))
        .reshape(1, D)
        .astype(bf),
        "bh": (np.asarray(b_h, dtype=np.float32) * 16.0).reshape(1, D).astype(bf),
    }

    in_maps = []
    for c in range(N_CORES):
        m = dict(common)
        m["x"] = x[c * B_CORE : (c + 1) * B_CORE]
        m["h"] = h_prev[c * B_CORE : (c + 1) * B_CORE]
        in_maps.append(m)

    LAST_RESULTS = run_bass_kernel_spmd(nc, in_maps, core_ids=list(range(N_CORES)))
    outs = LAST_RESULTS.results
    return np.concatenate([outs[c]["out"] for c in range(N_CORES)], axis=0)


# revision 29
# speedup vs baseline: 1.0136x; 1.0038x over previous
"""ImprovedGRUCell Trainium2 kernel v3 (8-core data-parallel over batch).

Design notes (vs the f32 baseline):
  - x / h_prev converted to bf16 on HOST -> input DMA traffic halved.
    h_prev is used in bf16 everywhere including the final blend.
  - fp8-e4m3 DoubleRow matmuls (full K=256 contraction in one pass, 0.5
    cycles/row) for the z-gate and attention-branch gemms; W_h / U_h
    (candidate state, the precision-critical path) stay bf16.
  - Softmax normalization: dB = ones128^T @ E8 (DoubleRow) broadcasts the
    per-column denominators to every PSUM partition; R = 1/dB (DVE),
    q = h^T * R, att = E8 * q -- so the U_h gemm consumes pre-normalized
    bf16 operands and accumulates into the SAME psum group as W_h's
    output; tanh reads one [128,1024] psum tile. No per-j scalar fixups.
  - 5-deep software pipeline over 512-row super-tiles (load+2, xpose+1,
    gemms+act, normalize+candidate-1, blend-2) so every in-order engine
    queue only ever waits on work from previous iterations.
  - GPSIMD (Pool) never touches PSUM (hardware restriction); all psum
    drains are DVE (2x mode for bf16) or ACT.
  - ACT does exactly 4 transcendental passes per super-tile with
    {Tanh, Exp} in one table set (z via 0.5*tanh(S/2)+0.5, the /2 folded
    into the activation scale).
  - PSUM: pz(2 banks) + pa(2) + pc(2) + transposes(2x1) = 8 banks.
"""

import os
import sys

sys.path.insert(0, "/opt/trn_rl_repo")

import ml_dtypes
import numpy as np

import concourse.bass as bass
import concourse.mybir as mybir
from concourse import bacc, tile
from concourse.bass_utils import run_bass_kernel_spmd

B_TOTAL = 65536
N_CORES = 8
B_CORE = B_TOTAL // N_CORES  # 8192
D = 256
ST = 512  # batch rows per super-tile
N_ST = B_CORE // ST  # 16

F32 = mybir.dt.float32
BF16 = mybir.dt.bfloat16
F8 = mybir.dt.float8e4
AF = mybir.ActivationFunctionType
ALU = mybir.AluOpType
DR = mybir.MatmulPerfMode.DoubleRow

# z-gate gemm precision: "full8" = both operands fp8 DoubleRow (fastest),
# "half" = x-side fp8-DR + h-side bf16 (better accuracy, PE still under the
# ACT bound), "bf16" = both sides bf16 (max accuracy, PE-bound)
ZMODE = os.environ.get("ZMODE", "half")
Z8 = ZMODE != "bf16"

_CACHE = {}


def build_nc(use_bias=False):
    nc = bacc.Bacc(
        "TRN2",
        target_bir_lowering=False,
        debug=False,
        enable_asserts=False,
        num_devices=N_CORES,
    )

    x_d = nc.dram_tensor("x", [B_CORE, D], BF16, kind="ExternalInput")
    h_d = nc.dram_tensor("h", [B_CORE, D], BF16, kind="ExternalInput")
    # weights in [128 p, kt 2, 256 h] contraction layout, host-packed
    wz_d = nc.dram_tensor("wz", [128, 2 * D], BF16, kind="ExternalInput")
    uz_d = nc.dram_tensor("uz", [128, 2 * D], BF16, kind="ExternalInput")
    wz8_d = nc.dram_tensor("wz8", [128, 2 * D], F8, kind="ExternalInput")
    uz8_d = nc.dram_tensor("uz8", [128, 2 * D], F8, kind="ExternalInput")
    wh_d = nc.dram_tensor("wh", [128, 2 * D], BF16, kind="ExternalInput")
    uh_d = nc.dram_tensor("uh", [128, 2 * D], BF16, kind="ExternalInput")
    uh8_d = nc.dram_tensor("uh8", [128, 2 * D], F8, kind="ExternalInput")
    wa_d = nc.dram_tensor("wa", [128, 2 * D], F8, kind="ExternalInput")
    ua_d = nc.dram_tensor("ua", [128, 2 * D], F8, kind="ExternalInput")
    va_d = nc.dram_tensor("va", [128, 2], F32, kind="ExternalInput")
    vad_d = nc.dram_tensor("vad", [128, 512], F8, kind="ExternalInput")
    id_d = nc.dram_tensor("ident", [128, 128], BF16, kind="ExternalInput")
    bz_d = nc.dram_tensor("bz", [1, D], BF16, kind="ExternalInput")
    bh_d = nc.dram_tensor("bh", [1, D], BF16, kind="ExternalInput")
    out_d = nc.dram_tensor("out", [B_CORE, D], F32, kind="ExternalOutput")

    with tile.TileContext(nc) as tc:
        with (
            tc.tile_pool(name="wp", bufs=1) as wp,
            tc.tile_pool(name="io", bufs=4) as io,
            tc.tile_pool(name="wk", bufs=3) as wk,
            tc.tile_pool(name="ppz", bufs=1, space="PSUM") as ppz,
            tc.tile_pool(name="ppa", bufs=1, space="PSUM") as ppa,
            tc.tile_pool(name="ppc", bufs=1, space="PSUM") as ppc,
            tc.tile_pool(name="pxp", bufs=2, space="PSUM") as pxp,
        ):
            # ---- persistent weights -------------------------------------
            # (ident + the first two activation loads are emitted before the
            # weight DMAs so the transpose pipeline fills ASAP; see below)
            def wload(name, dram, dt):
                t = wp.tile([128, 2 * D], dt, tag=name)
                nc.sync.dma_start(out=t[:], in_=dram.ap())
                return t.rearrange("p (kt h) -> p kt h", kt=2)

            ident = wp.tile([128, 128], BF16, tag="ident")
            nc.sync.dma_start(out=ident[:], in_=id_d.ap())
            ones8 = wp.tile([128, 2 * 128], F8, tag="ones8")
            nc.vector.memset(ones8[:], 1.0 / 16)
            ones8v = ones8.rearrange("p (kt m) -> p kt m", kt=2)
            if use_bias:
                bz = wp.tile([1, D], BF16, tag="bz")
                nc.sync.dma_start(out=bz[:], in_=bz_d.ap())
                bh = wp.tile([1, D], BF16, tag="bh")
                nc.sync.dma_start(out=bh[:], in_=bh_d.ap())
                ones_r = wp.tile([1, 128], BF16, tag="ones_r")
                nc.vector.memset(ones_r[:], 1.0)

            S = {}  # per-super-tile tile handles across pipeline stages

            def load(i):
                b0 = i * ST
                xb = io.tile([128, 4 * D], BF16, tag="xb", bufs=3)
                nc.sync.dma_start(
                    out=xb.rearrange("p (j k) -> p j k", j=4),
                    in_=x_d.ap()[b0 : b0 + ST, :].rearrange("(j p) k -> p j k", p=128),
                )
                hb = io.tile([128, 4 * D], BF16, tag="hb", bufs=7)
                nc.sync.dma_start(
                    out=hb.rearrange("p (j k) -> p j k", j=4),
                    in_=h_d.ap()[b0 : b0 + ST, :].rearrange("(j p) k -> p j k", p=128),
                )
                S[i] = dict(xb=xb, hb=hb)

            def xpose(i):
                # [p=b, j, k] -> [p=k, kt, b]  (kt-major, DoubleRow layout)
                s = S[i]

                def one(src3, tag):
                    pt = pxp.tile([128, 4 * D], BF16, tag="pxp")
                    pt3 = pt.rearrange("p (kt b) -> p kt b", kt=2)
                    for kt in range(2):
                        for j in range(4):
                            nc.tensor.transpose(
                                pt3[:, kt, j * 128 : (j + 1) * 128],
                                src3[:, j, kt * 128 : (kt + 1) * 128],
                                ident[:],
                            )
                    t16 = wk.tile([128, 4 * D], BF16, tag=tag, bufs=6)
                    nc.vector.tensor_copy(t16[:], pt[:])
                    t8 = wk.tile([128, 4 * D], F8, tag=tag + "8", bufs=4)
                    nc.gpsimd.tensor_copy(t8[:], t16[:])
                    return t16, t8

                s["xT"], s["xT8"] = one(s["xb"].rearrange("p (j k) -> p j k", j=4), "xT")
                s["hT"], s["hT8"] = one(s["hb"].rearrange("p (j k) -> p j k", j=4), "hT")

            def gemmsA(i):
                s = S[i]
                xT83 = s["xT8"].rearrange("p (kt b) -> p kt b", kt=2)
                hT83 = s["hT8"].rearrange("p (kt b) -> p kt b", kt=2)

                # ---- z-gate gemm: psum = S_z (+ b_z) --------------------
                pz = ppz.tile([128, 4 * D], F32, tag="ppz")
                pz3 = pz.rearrange("p (j h) -> p j h", j=4)
                zmode_i = "bf16" if (ZMODE == "half" and i == 0) else ZMODE
                if zmode_i == "full8":
                    for j in range(4):
                        nc.tensor.matmul(
                            pz3[:, j],
                            xT83[:, :, j * 128 : (j + 1) * 128],
                            wz8[:, :, :],
                            start=True,
                            stop=False,
                            perf_mode=DR,
                        )
                        nc.tensor.matmul(
                            pz3[:, j],
                            hT83[:, :, j * 128 : (j + 1) * 128],
                            uz8[:, :, :],
                            start=False,
                            stop=not use_bias,
                            perf_mode=DR,
                            skip_group_check=True,
                        )
                        if use_bias:
                            nc.tensor.matmul(
                                pz3[:, j], ones_r[:], bz[:], start=False, stop=True,
                                skip_group_check=True,
                            )
                elif zmode_i == "half":
                    # x-side fp8-DR (weights host-scaled x8); h-side bf16.
                    # bf16 U_z is host-scaled x8 too so both sides share the
                    # tanh input scale of 0.5/8.
                    hT3h = s["hT"].rearrange("p (kt b) -> p kt b", kt=2)
                    for j in range(4):
                        nc.tensor.matmul(
                            pz3[:, j],
                            xT83[:, :, j * 128 : (j + 1) * 128],
                            wz8[:, :, :],
                            start=True,
                            stop=False,
                            perf_mode=DR,
                        )
                        for kt in range(2):
                            nc.tensor.matmul(
                                pz3[:, j],
                                hT3h[:, kt, j * 128 : (j + 1) * 128],
                                uz[:, kt],
                                start=False,
                                stop=(kt == 1 and not use_bias),
                                skip_group_check=True,
                            )
                        if use_bias:
                            nc.tensor.matmul(
                                pz3[:, j], ones_r[:], bz[:], start=False, stop=True,
                                skip_group_check=True,
                            )
                else:
                    xT3 = s["xT"].rearrange("p (kt b) -> p kt b", kt=2)
                    hT3 = s["hT"].rearrange("p (kt b) -> p kt b", kt=2)
                    n_pass = 5 if use_bias else 4
                    for j in range(4):
                        k = 0
                        for src, w3 in ((xT3, wz), (hT3, uz)):
                            for kt in range(2):
                                nc.tensor.matmul(
                                    pz3[:, j],
                                    src[:, kt, j * 128 : (j + 1) * 128],
                                    w3[:, kt],
                                    start=(k == 0),
                                    stop=(k == n_pass - 1),
                                )
                                k += 1
                        if use_bias:
                            nc.tensor.matmul(
                                pz3[:, j], ones_r[:], bz[:], start=False, stop=True
                            )
                # t = tanh((S_z+b_z)/2); z = 0.5*t+0.5 later
                t_t = wk.tile([128, 4 * D], BF16, tag="t", bufs=5)
                # fp8 weights are host-scaled by 8 (keeps them in e4m3's
                # normal range); fold the 1/8 into the tanh input scale
                nc.scalar.activation(
                    t_t[:], pz[:], AF.Tanh, scale=(0.5 / 8 if Z8 else 0.5)
                )

                # ---- attention gemms (fp8 DoubleRow), transposed --------
                pa = ppa.tile([128, 4 * D], F32, tag="ppa")
                pa3 = pa.rearrange("p (ht b) -> p ht b", ht=2)
                for ht in range(2):
                    nc.tensor.matmul(
                        pa3[:, ht],
                        wa[:, :, ht * 128 : (ht + 1) * 128],
                        xT83[:, :, :],
                        start=True,
                        stop=False,
                        perf_mode=DR,
                    )
                    nc.tensor.matmul(
                        pa3[:, ht],
                        ua[:, :, ht * 128 : (ht + 1) * 128],
                        hT83[:, :, :],
                        start=False,
                        stop=True,
                        perf_mode=DR,
                    )
                A = wk.tile([128, 4 * D], F8, tag="A", bufs=2)
                A3 = A.rearrange("p (ht b) -> p ht b", ht=2)
                nc.scalar.activation(A[:], pa[:], AF.Tanh, scale=1.0 / 8)
                E8 = wk.tile([128, 4 * D], F8, tag="E8", bufs=3)
                if i <= 1:
                    # during pipeline fill the ACT->PE->ACT round trip below
                    # would stall ACT; use the direct per-ht exp
                    for ht in range(2):
                        nc.scalar.activation(
                            E8[:, ht * 512 : (ht + 1) * 512],
                            A[:, ht * 512 : (ht + 1) * 512],
                            AF.Exp,
                            scale=va[:, ht : ht + 1],
                        )
                else:
                    # u = diag(va) (x) A via fp8 DoubleRow, overwriting pa;
                    # then a single full-width exp
                    for ht in range(2):
                        nc.tensor.matmul(
                            pa3[:, ht],
                            vad4[:, ht],
                            A3[:, :, :],
                            start=True,
                            stop=True,
                            perf_mode=DR,
                            skip_group_check=True,
                        )
                    nc.scalar.activation(E8[:], pa[:], AF.Exp)
                s.update(E8=E8, t=t_t)

            def stageB1(i):
                s = S[i]
                # broadcast softmax denominators: every psum partition gets
                # d[b] = sum_h E8[h,b]; lives in the transpose-psum ring,
                # which is idle at this point of the iteration
                dBt = pxp.tile([128, 512], F32, tag="pxp")
                dB = dBt[:, :]
                E83 = s["E8"].rearrange("p (ht b) -> p ht b", ht=2)
                nc.tensor.matmul(
                    dB,
                    ones8v[:, :, :],
                    E83[:, :, :],
                    start=True,
                    stop=True,
                    perf_mode=DR,
                    skip_group_check=True,
                )
                R = wk.tile([128, 512], BF16, tag="R", bufs=2)
                with nc.allow_low_precision(reason="1/denominator to bf16 is fine"):
                    nc.vector.reciprocal(R[:], dB)
                q = wk.tile([128, 4 * D], BF16, tag="q", bufs=2)
                q3 = q.rearrange("p (kt b) -> p kt b", kt=2)
                hT3 = s["hT"].rearrange("p (kt b) -> p kt b", kt=2)
                att = wk.tile([128, 4 * D], F8, tag="att", bufs=3)
                # att^T = E8 * (h^T / d)  -- normalized, bf16.  The last
                # tile runs in b-halves so the epilogue chain pipelines.
                halves = 2 if i == N_ST - 1 else 1
                bw = 512 // halves
                for hh in range(halves):
                    bsl = slice(hh * bw, (hh + 1) * bw)
                    for kt in range(2):
                        nc.vector.tensor_mul(
                            q3[:, kt, bsl], hT3[:, kt, bsl], R[:, bsl]
                        )
                    if halves == 1:
                        nc.gpsimd.tensor_mul(att[:], s["E8"][:], q[:])
                    else:
                        att3 = att.rearrange("p (kt b) -> p kt b", kt=2)
                        E83f = s["E8"].rearrange("p (kt b) -> p kt b", kt=2)
                        for kt in range(2):
                            nc.gpsimd.tensor_mul(
                                att3[:, kt, bsl], E83f[:, kt, bsl], q3[:, kt, bsl]
                            )
                s["att"] = att

            def stageB2(i):
                s = S[i]
                att3 = s["att"].rearrange("p (kt b) -> p kt b", kt=2)
                xT3 = s["xT"].rearrange("p (kt b) -> p kt b", kt=2)

                # candidate: psum = x@W_h^T + att@U_h^T (+ b_h), per-j group
                # (the final tile borrows the z psum banks, idle by then, so
                # the last two candidate gemms don't serialize on one ring)
                pool_c = ppz if i == N_ST - 1 else ppc
                tag_c = "ppz" if i == N_ST - 1 else "ppc"
                pc = pool_c.tile([128, 4 * D], F32, tag=tag_c)
                pc3 = pc.rearrange("p (j h) -> p j h", j=4)
                for j in range(4):
                    for kt in range(2):
                        nc.tensor.matmul(
                            pc3[:, j],
                            xT3[:, kt, j * 128 : (j + 1) * 128],
                            wh[:, kt],
                            start=(kt == 0),
                            stop=False,
                        )
                    nc.tensor.matmul(
                        pc3[:, j],
                        att3[:, :, j * 128 : (j + 1) * 128],
                        uh8[:, :, :],
                        start=False,
                        stop=not use_bias,
                        perf_mode=DR,
                        skip_group_check=True,
                    )
                    if use_bias:
                        nc.tensor.matmul(
                            pc3[:, j], ones_r[:], bh[:], start=False, stop=True,
                            skip_group_check=True,
                        )
                htl = wk.tile([128, 4 * D], BF16, tag="htl", bufs=3)
                if i == N_ST - 1:
                    nc.scalar.activation(htl[:, 0:512], pc[:, 0:512], AF.Tanh, scale=1 / 16)
                    nc.scalar.activation(htl[:, 512:1024], pc[:, 512:1024], AF.Tanh, scale=1 / 16)
                else:
                    nc.scalar.activation(htl[:], pc[:], AF.Tanh, scale=1 / 16)
                s["htl"] = htl

            def stageC(i):
                s = S.pop(i)
                b0 = i * ST
                zb = wk.tile([128, 4 * D], BF16, tag="zb")
                if i != N_ST - 1:
                    nc.vector.tensor_scalar(
                        zb[:], s["t"][:], 0.5, 0.5, op0=ALU.mult, op1=ALU.add
                    )
                ot = io.tile([128, 4 * D], F32, tag="ot", bufs=3)
                # the last tile drains in halves so DVE/Pool/DMA pipeline
                # behind the final tanh instead of serializing the epilogue
                halves = 2 if i == N_ST - 1 else 1
                w = (4 * D) // halves
                rows = ST // halves
                for hh in range(halves):
                    sl = slice(hh * w, (hh + 1) * w)
                    if i == N_ST - 1:
                        nc.vector.tensor_scalar(
                            zb[:, sl], s["t"][:, sl], 0.5, 0.5,
                            op0=ALU.mult, op1=ALU.add,
                        )
                    d = wk.tile([128, w], BF16, tag=f"d{hh}")
                    nc.vector.tensor_sub(d[:], s["htl"][:, sl], s["hb"][:, sl])
                    p = wk.tile([128, w], BF16, tag=f"p{hh}")
                    nc.vector.tensor_mul(p[:], zb[:, sl], d[:])
                    nc.gpsimd.tensor_add(ot[:, sl], p[:], s["hb"][:, sl])
                    r0 = b0 + hh * rows
                    nc.sync.dma_start(
                        out=out_d.ap()[r0 : r0 + rows, :].rearrange(
                            "(j p) k -> p j k", p=128
                        ),
                        in_=ot[:, sl].rearrange("p (j k) -> p j k", j=4 // halves),
                    )

            load(0)
            # wz/uz gate tile 0's bf16 z-gemm (the first ACT op): load them
            # right after tile 0's data, before the tile-1 prefetch (which
            # isn't consumed until a full iteration later)
            wz = wload("wz", wz_d, BF16) if ZMODE in ("bf16", "half") else None
            uz = wload("uz", uz_d, BF16) if ZMODE in ("bf16", "half") else None
            if N_ST > 1:
                load(1)
            xpose(0)
            wz8 = wload("wz8", wz8_d, F8) if Z8 else None
            uz8 = wload("uz8", uz8_d, F8) if ZMODE == "full8" else None
            wh = wload("wh", wh_d, BF16)  # host-scaled x16
            uh8 = wload("uh8", uh8_d, F8)
            wa = wload("wa", wa_d, F8)
            ua = wload("ua", ua_d, F8)
            vad = wp.tile([128, 512], F8, tag="vad")
            nc.sync.dma_start(out=vad[:], in_=vad_d.ap())
            vad4 = vad.rearrange("p (sel t m) -> p sel t m", sel=2, t=2)
            va = wp.tile([128, 2], F32, tag="va")
            nc.sync.dma_start(out=va[:], in_=va_d.ap())
            if N_ST > 1:
                xpose(1)
            for it in range(N_ST + 3):
                if it + 2 < N_ST:
                    load(it + 2)
                if it < N_ST:
                    gemmsA(it)
                if 0 <= it - 2 < N_ST:
                    stageB2(it - 2)
                if it + 2 < N_ST:
                    xpose(it + 2)
                if it < N_ST:
                    stageB1(it)
                if 0 <= it - 3 < N_ST:
                    stageC(it - 3)

    nc.compile()
    return nc


LAST_RESULTS = None


def _pack_vad(v_a):
    # diag(va) as DoubleRow lhsT pair: [p, sel(2), t(2), m(128)];
    # sel=ht selects which half of va sits on the diagonal (t == sel)
    f8 = ml_dtypes.float8_e4m3
    v = np.asarray(v_a, dtype=np.float32)
    Dv = np.zeros((128, 2, 2, 128), dtype=np.float32)
    idx = np.arange(128)
    Dv[idx, 0, 0, idx] = v[:128]
    Dv[idx, 1, 1, idx] = v[128:]
    return np.ascontiguousarray(Dv.reshape(128, 512).astype(f8))


def _pack_w(W, dt):
    # [out,in] weight -> contraction layout [128 p, kt 2, 256 out] flat
    WT = np.asarray(W, dtype=np.float32).T  # [in 256, out 256]
    return np.ascontiguousarray(
        WT.reshape(2, 128, D).transpose(1, 0, 2).reshape(128, 2 * D).astype(dt)
    )


def kernel(x, h_prev, W_z, U_z, b_z, W_a, U_a, v_a, W_h, U_h, b_h):
    global LAST_RESULTS
    use_bias = bool(np.any(np.asarray(b_z)) or np.any(np.asarray(b_h)))
    key = ("nc", use_bias)
    if key not in _CACHE:
        _CACHE[key] = build_nc(use_bias)
    nc = _CACHE[key]

    bf = ml_dtypes.bfloat16
    f8 = ml_dtypes.float8_e4m3
    x = np.asarray(x, dtype=np.float32).astype(bf)
    h_prev = np.asarray(h_prev, dtype=np.float32).astype(bf)

    common = {
        "wz": _pack_w(
            np.asarray(W_z) * (8.0 if ZMODE == "half" else 1.0), bf
        ),
        "uz": _pack_w(
            np.asarray(U_z) * (8.0 if ZMODE == "half" else 1.0), bf
        ),
        "wz8": _pack_w(np.asarray(W_z) * 8.0, f8),
        "uz8": _pack_w(np.asarray(U_z) * 8.0, f8),
        "wh": _pack_w(np.asarray(W_h) * 16.0, bf),
        "uh": _pack_w(U_h, bf),
        "uh8": _pack_w(U_h, f8),
        "wa": _pack_w(np.asarray(W_a) * 8.0, f8),
        "ua": _pack_w(np.asarray(U_a) * 8.0, f8),
        "va": np.ascontiguousarray(
            np.asarray(v_a, dtype=np.float32).reshape(2, 128).T
        ),
        "vad": _pack_vad(v_a),
        "ident": np.eye(128, dtype=bf),
        "bz": (np.asarray(b_z, dtype=np.float32) * (8.0 if Z8 else 1.0))
        .reshape(1, D)
        .astype(bf),
        "bh": (np.asarray(b_h, dtype=np.float32) * 16.0).reshape(1, D).astype(bf),
    }

    in_maps = []
    for c in range(N_CORES):
        m = dict(common)
        m["x"] = x[c * B_CORE : (c + 1) * B_CORE]
        m["h"] = h_prev[c * B_CORE : (c + 1) * B_CORE]
        in_maps.append(m)

    LAST_RESULTS = run_bass_kernel_spmd(nc, in_maps, core_ids=list(range(N_CORES)))
    outs = LAST_RESULTS.results
    return np.concatenate([outs[c]["out"] for c in range(N_CORES)], axis=0)
